# revision 10
# baseline (speedup 1.0000x reference)
"""Trainium2 Bass kernel for nn_MultiHeadCrossAttention (B=16, Dq=768, H=12,
hd=64, Nq=1024, Nt=64, Dkv=384) with RoPE on q and k.

Sharding: pure data-parallel over batch, 2 batches per core across 8 cores.
No collectives.

v3 design (per-core, channel-major):
  qproj:  fp8e4 DoubleRow matmuls (Wq x64 host-scaled, feat in fp8): 2 K-tiles
          per instruction.
  q-RoPE: qc = qpsum * (cos*scale/64) [DVE, ->fp8], qs = qc * tan [Pool, fp8].
          qc/qs interleaved in one tile = the 2 K-subtiles of a DoubleRow
          scores matmul.
  k side: bf16 RoPE folded into kA/kB, written directly into block-diagonal
          fp8 tiles (kabblk): ONE DoubleRow matmul per head-pair computes
          kA.T@qc + kB.T@qs for both heads.
  E = exp(scores) -> fp8 [ACT]. Denominators: DoubleRow over pair-pairs with
          an fp8 0/1 selection lhsT. recip on DVE -> f32 r [12,512].
  r broadcast: DRAM round-trip with a stride-0 AP (no PE/copy cost, DMA only).
  AV:     block-diagonal fp8 v (vblk): 1 matmul per pair.
  attnT = av * rhat -> fp8 [DVE] (normalize + fp8 convert fused)
  oproj:  fp8 DoubleRow (Wout x16, v x4); residual (feat+bout)*64 bf16 is
          accumulated into the same PSUM group via an identity matmul; PSUM ->
          bf16 out copies on ACT. Host divides by 64.
"""

import os
import sys
from contextlib import ExitStack

import numpy as np

sys.path.insert(0, "/opt/trn_rl_repo")

import concourse.bass as bass  # noqa: E402
import concourse.mybir as mybir  # noqa: E402
import concourse.tile as tile  # noqa: E402
from concourse import bacc  # noqa: E402
from concourse.bass_utils import run_bass_kernel_spmd  # noqa: E402

import ml_dtypes

F32 = mybir.dt.float32
BF16 = mybir.dt.bfloat16
F8 = mybir.dt.float8e4
NPBF = ml_dtypes.bfloat16
NPF8 = ml_dtypes.float8_e4m3
DRMODE = mybir.MatmulPerfMode.DoubleRow

B, DQ, T, HP, WP = 16, 768, 4, 16, 16
NQ = T * HP * WP            # 1024
NT, DKV = 64, 384
H, HD = 12, 64
SCALE = HD ** -0.5
NCORES = 8
BL = B // NCORES            # batches per core = 2
CHUNK = 512                 # query positions per chunk
NCH = NQ // CHUNK           # chunks per batch = 2
KQ = DQ // 128              # 6 contraction tiles for Dq
KKV = DKV // 128            # 3 contraction tiles for Dkv
NPAIR = H // 2              # 6 head pairs
SW_Q = 64.0                 # Wq host scale (fp8 range); undone in cq table
SW_V = 4.0                  # Wv host scale (attnT fp8 range)
SW_O = 16.0                 # Wout host scale (fp8 range)
SW_VO = SW_V * SW_O         # total out scale; undone on host


def _rope_tables(n):
    inv_freq = 1.0 / (10000.0 ** (np.arange(0, HD, 2, dtype=np.float64) / HD))
    freqs = np.arange(n, dtype=np.float64)[:, None] * inv_freq[None, :]
    emb = np.concatenate([freqs, freqs], axis=-1)  # [n, 64]
    return np.cos(emb).T, np.sin(emb).T            # [64, n] f64


def _consts():
    cq, sq = _rope_tables(NQ)          # [64, 1024] f64
    ck, sk = _rope_tables(NT)          # [64, 64]
    # q tables: RoPE scale and the 1/SW_Q unscale folded in; tan trick for qs
    cq2 = np.ascontiguousarray(
        np.tile(cq * SCALE / SW_Q, (2, 1))).astype(NPBF)       # [128, 1024]
    tanq2 = np.ascontiguousarray(np.tile(sq / cq, (2, 1))).astype(NPBF)
    # k tables: duplicated 2 heads (partitions), tiled (KQ, BL) along free to
    # match kT's [128, KQ, BL, 64] free layout
    ck2 = np.ascontiguousarray(np.tile(ck, (2, 2 * KQ))).astype(np.float32)
    sk2 = np.ascontiguousarray(np.tile(sk, (2, 2 * KQ))).astype(np.float32)
    eps = np.where(np.arange(HD) < HD // 2, -1.0, 1.0).astype(np.float32)
    epsv = np.ascontiguousarray(np.tile(eps, 2)[:, None])      # [128, 1]
    ident = np.eye(128, dtype='float32')
    # denominator lhsT: for pair j, col 2j sums partitions 0-63 (even head),
    # col 2j+1 sums partitions 64-127 (odd head). fp8 (0/1 exact). Padded to
    # 16 cols: dual-fp8 ldweights needs a 16B-aligned subtile stride.
    dlhs = np.zeros((128, NPAIR, 16), np.float32)
    for j in range(NPAIR):
        dlhs[:64, j, 2 * j] = 1.0
        dlhs[64:, j, 2 * j + 1] = 1.0
    return dict(cq=cq2, tanq=tanq2, ck=ck2, sk=sk2, epsv=epsv, nepsv=-epsv,
                ident=ident.astype(NPBF), dlhs=dlhs.astype(NPF8))


def _sigma_dma(nc, out_ap, in_ap):
    """out = in with 32-partition halves swapped inside each 64 block."""
    for dst, src in ((0, 32), (32, 0), (64, 96), (96, 64)):
        nc.gpsimd.dma_start(out=out_ap[dst:dst + 32], in_=in_ap[src:src + 32])


def build(debug=False):
    nc = bacc.Bacc(None, target_bir_lowering=False, debug=debug)
    with tile.TileContext(nc) as tc:
        with tc.tile_pool(name="dram", bufs=1, space="DRAM") as dram:
            def din(name, shape, dt=F32):
                return dram.tile(shape, dt, kind="ExternalInput", name=name,
                                 uniquify=False)

            featb = din("featb", [BL, 128, KQ, NQ], BF16)
            feat8 = din("feat8", [BL, 128, KQ, NQ], F8)
            tok_l = din("tok_l", [BL * NT, DKV], BF16)
            wq8 = din("wq8", [128, KQ, DQ], F8)
            wk = din("wk", [128, KKV, DQ], BF16)
            wv = din("wv", [128, KKV, DQ], BF16)
            wo8 = din("wo8", [128, KQ, DQ], F8)
            cq = din("cq", [128, NQ], BF16)
            tanq = din("tanq", [128, NQ], BF16)
            ck = din("ck", [128, KQ * 128])
            sk = din("sk", [128, KQ * 128])
            epsv = din("epsv", [128, 1])
            nepsv = din("nepsv", [128, 1])
            ident = din("ident", [128, 128], BF16)
            dlhs = din("dlhs", [128, NPAIR, 16], F8)
            r_dram = dram.tile([2, H, CHUNK], F32, name="r_scratch")
            out_l = dram.tile([BL, 128, KQ, NQ], BF16, kind="ExternalOutput",
                              name="out_l", uniquify=False)

            with ExitStack() as body_ctx:
                global _body_ctx
                _body_ctx = body_ctx
                _body(nc, tc, featb, feat8, tok_l, wq8, wk, wv, wo8,
                      cq, tanq, ck, sk, epsv, nepsv, ident, dlhs, r_dram,
                      out_l)
    nc.compile()
    return nc


def _body(nc, tc, featb, feat8, tok_l, wq8, wk, wv, wo8, cq, tanq, ck,
          sk, epsv, nepsv, ident, dlhs, r_dram, out_l):
    MULT = mybir.AluOpType.mult
    ADD = mybir.AluOpType.add
    EXP = mybir.ActivationFunctionType.Exp

    ctx = _body_ctx
    consts = ctx.enter_context(tc.tile_pool(name="consts", bufs=1))
    kside = ctx.enter_context(tc.tile_pool(name="kside", bufs=1))
    ktmp = ctx.enter_context(tc.tile_pool(name="ktmp", bufs=1))
    featp = ctx.enter_context(tc.tile_pool(name="featp", bufs=2))
    qp = ctx.enter_context(tc.tile_pool(name="qp", bufs=2))
    ep = ctx.enter_context(tc.tile_pool(name="ep", bufs=2))
    atp = ctx.enter_context(tc.tile_pool(name="atp", bufs=2))
    outp = ctx.enter_context(tc.tile_pool(name="outp", bufs=2))
    rp = ctx.enter_context(tc.tile_pool(name="rp", bufs=2))

    pp = ctx.enter_context(tc.tile_pool(name="pp", bufs=3, space="PSUM"))
    attn = ctx.enter_context(tc.tile_pool(name="attn", bufs=4, space="PSUM"))
    dp = ctx.enter_context(tc.tile_pool(name="dp", bufs=1, space="PSUM"))

    # ---- load constants. sync ring carries the qproj/phase0 critical path.
    wq_sb = consts.tile([128, KQ, DQ], F8)
    nc.sync.dma_start(out=wq_sb, in_=wq8[:])
    tok_sb = consts.tile([128, DKV], BF16)
    nc.sync.dma_start(out=tok_sb, in_=tok_l[:])
    id_sb = consts.tile([128, 128], BF16)
    nc.sync.dma_start(out=id_sb, in_=ident[:])
    wk_sb = consts.tile([128, KKV, DQ], BF16)
    nc.sync.dma_start(out=wk_sb, in_=wk[:])
    cq_sb = consts.tile([128, NQ], BF16)
    nc.sync.dma_start(out=cq_sb, in_=cq[:])
    tanq_sb = consts.tile([128, NQ], BF16)
    nc.scalar.dma_start(out=tanq_sb, in_=tanq[:])
    wv_sb = consts.tile([128, KKV, DQ], BF16)
    nc.scalar.dma_start(out=wv_sb, in_=wv[:])
    ck_sb = consts.tile([128, KQ * 128], F32)
    nc.scalar.dma_start(out=ck_sb, in_=ck[:])
    sk_sb = consts.tile([128, KQ * 128], F32)
    nc.scalar.dma_start(out=sk_sb, in_=sk[:])
    eps_sb = consts.tile([128, 1], F32)
    nc.scalar.dma_start(out=eps_sb, in_=epsv[:])
    neps_sb = consts.tile([128, 1], F32)
    nc.scalar.dma_start(out=neps_sb, in_=nepsv[:])
    dlhs_sb = consts.tile([128, NPAIR, 16], F8)
    nc.scalar.dma_start(out=dlhs_sb, in_=dlhs[:])
    wo_sb = consts.tile([128, KQ, DQ], F8)
    nc.scalar.dma_start(out=wo_sb, in_=wo8[:])

    # ---- phase 0: tokensT, kT, k-RoPE into blockdiag kabblk, v/vblk ----
    _ph0 = nc.named_scope("ph0")
    _ph0.__enter__()
    tokT_sb = kside.tile([128, KKV, 128], BF16)
    for ct in range(KKV):
        tp = pp.tile([128, 128], BF16, tag="pp")
        nc.tensor.transpose(tp, tok_sb[:, ct * 128:(ct + 1) * 128], id_sb[:])
        nc.scalar.copy(out=tokT_sb[:, ct, :], in_=tp)

    # kT free layout: (pair, batch, token)
    kT_sb = kside.tile([128, KQ, BL, 64], F32)
    for m in range(KQ):
        kp = pp.tile([128, 128], F32, tag="pp")
        for kc in range(KKV):
            nc.tensor.matmul(kp, wk_sb[:, kc, m * 128:(m + 1) * 128],
                             tokT_sb[:, kc, :],
                             start=(kc == 0), stop=(kc == KKV - 1))
        nc.scalar.copy(out=kT_sb[:, m, :, :],
                       in_=kp.rearrange("p (b t) -> p b t", b=BL))

    # blockdiag kA/kB in fp8: kabblk[p, j, b, 0/1, :]; even head top-left
    # 64x64, odd head bottom-right. Off-diagonal quarters stay zero.
    kabblk = kside.tile([128, KQ, BL, 2, 128], F8)
    nc.scalar.memzero(kabblk[:])
    t1 = ktmp.tile([128, KQ, BL, 64], F32, tag="t1")
    t2 = ktmp.tile([128, KQ, BL, 64], F32, tag="t2")
    t1s = ktmp.tile([128, KQ, BL, 64], F32, tag="t1s")
    t2s = ktmp.tile([128, KQ, BL, 64], F32, tag="t2s")
    nc.gpsimd.tensor_mul(t1, kT_sb[:], ck_sb[:].rearrange(
        "p (j b t) -> p j b t", j=KQ, b=BL))
    nc.gpsimd.tensor_mul(t2, kT_sb[:], sk_sb[:].rearrange(
        "p (j b t) -> p j b t", j=KQ, b=BL))
    _sigma_dma(nc, t1s, t1)
    _sigma_dma(nc, t2s, t2)
    # kA = k_rot = t1 + eps*sigma(t2);  kB = t2 - eps*sigma(t1)
    nc.vector.scalar_tensor_tensor(
        out=kabblk[0:64, :, :, 0, 0:64], in0=t2s[0:64], scalar=eps_sb[0:64],
        in1=t1[0:64], op0=MULT, op1=ADD)
    nc.vector.scalar_tensor_tensor(
        out=kabblk[64:128, :, :, 0, 64:128], in0=t2s[64:128],
        scalar=eps_sb[64:128], in1=t1[64:128], op0=MULT, op1=ADD)
    nc.vector.scalar_tensor_tensor(
        out=kabblk[0:64, :, :, 1, 0:64], in0=t1s[0:64], scalar=neps_sb[0:64],
        in1=t2[0:64], op0=MULT, op1=ADD)
    nc.vector.scalar_tensor_tensor(
        out=kabblk[64:128, :, :, 1, 64:128], in0=t1s[64:128],
        scalar=neps_sb[64:128], in1=t2[64:128], op0=MULT, op1=ADD)

    # v, natural [token, dim] layout, duplicated across partition halves,
    # viewed as [128, b, pair, half, 64]
    vv_sb = kside.tile([128, BL, NPAIR, 2, 64], BF16)
    for b in range(BL):
        for nn in range(2):   # 2 chunks of 3 pairs = 384 dims
            vps = pp.tile([128, 384], F32, tag="pp")
            for half in range(2):
                for kc in range(KKV):
                    nc.tensor.matmul(
                        vps[64 * half:64 * half + 64, :],
                        tokT_sb[:, kc, b * 64:(b + 1) * 64],
                        wv_sb[:, kc, nn * 384:(nn + 1) * 384],
                        start=(kc == 0), stop=(kc == KKV - 1))
            nc.scalar.copy(
                out=vv_sb[:, b, 3 * nn:3 * nn + 3, :, :],
                in_=vps.rearrange("p (j h t) -> p j h t", j=3, h=2))

    vblk = kside.tile([128, BL, NPAIR, 128], F8)
    nc.scalar.memzero(vblk[:])
    nc.scalar.copy(out=vblk[0:64, :, :, 0:64], in_=vv_sb[0:64, :, :, 0, :])
    nc.scalar.copy(out=vblk[64:128, :, :, 64:128],
                   in_=vv_sb[64:128, :, :, 1, :])

    _ph0.__exit__(None, None, None)

    # ---- main loop: software-pipelined across the 4 (batch, chunk) steps ----
    chunks = [(b, c) for b in range(BL) for c in range(NCH)]
    st = {}

    def stage_qproj(i):
        b, c = chunks[i]
        p0 = c * CHUNK
        f8b = featp.tile([128, KQ, CHUNK], F8, tag="f8", name=f"f8_{i}")
        nc.sync.dma_start(out=f8b, in_=feat8[b, :, :, p0:p0 + CHUNK])
        fbb = featp.tile([128, KQ, CHUNK], BF16, tag="fb", name=f"fb{i}")
        nc.sync.dma_start(out=fbb, in_=featb[b, :, :, p0:p0 + CHUNK])
        # qc/qs interleaved: the 2 K-subtiles of the DoubleRow scores matmul
        qcs = qp.tile([128, KQ, 2, CHUNK], F8, tag="qcs", name=f"qcs{i}")
        for m in range(KQ):
            qps = pp.tile([128, CHUNK], F32, tag="pp", name=f"qp{i}_{m}")
            for kc in range(KQ // 2):
                nc.tensor.matmul(qps,
                                 wq_sb[:, 2 * kc:2 * kc + 2,
                                       m * 128:(m + 1) * 128],
                                 f8b[:, 2 * kc:2 * kc + 2, :],
                                 start=(kc == 0), stop=(kc == KQ // 2 - 1),
                                 perf_mode=DRMODE)
            nc.vector.tensor_mul(qcs[:, m, 0, :], qps, cq_sb[:, p0:p0 + CHUNK])
            nc.gpsimd.tensor_mul(qcs[:, m, 1, :], qcs[:, m, 0, :],
                                 tanq_sb[:, p0:p0 + CHUNK])
        st[i] = dict(qcs=qcs, fb=fbb)

    def stage_qk(i):
        b, c = chunks[i]
        s = st[i]
        qcs = s["qcs"]
        e_sb = ep.tile([128, NPAIR, CHUNK], F8, tag="e", name=f"e{i}")
        dps = dp.tile([16, CHUNK], F32, tag="den", name=f"d{i}")

        def qk1(j):
            sps = attn.tile([128, CHUNK], F32, tag="attn", name=f"s{i}_{j}")
            nc.tensor.matmul(sps, kabblk[:, j, b, :, :], qcs[:, j, :, :],
                             start=True, stop=True, perf_mode=DRMODE)
            nc.scalar.activation(out=e_sb[:, j, :], in_=sps, func=EXP)

        def denom(t):
            nc.tensor.matmul(dps, dlhs_sb[:, 2 * t:2 * t + 2, :],
                             e_sb[:, 2 * t:2 * t + 2, :],
                             start=(t == 0), stop=(t == NPAIR // 2 - 1),
                             perf_mode=DRMODE)

        for j in range(NPAIR):
            qk1(j)
            if j % 2 == 1 and j >= 3:
                denom(j // 2 - 1)
        denom(NPAIR // 2 - 1)
        s["e"], s["dps"] = e_sb, dps

    def stage_recip(i):
        s = st[i]
        r32 = rp.tile([H, CHUNK], F32, tag="r32", name=f"r32_{i}")
        nc.vector.reciprocal_approx_fast(out=r32, in_=s["dps"][0:H])
        # broadcast r to 128 partitions via DRAM round-trip (stride-0 read):
        # rhat[p, j, q] = r[2j + (p>=64), q]
        slot = i % 2
        nc.sync.dma_start(out=r_dram[slot], in_=r32)
        rhat = rp.tile([128, NPAIR, CHUNK], F32, tag="rhat", name=f"rh{i}")
        src = r_dram[slot].rearrange("(j two) q -> j two q", two=2)
        for half in range(2):
            nc.sync.dma_start(
                out=rhat[64 * half:64 * half + 64],
                in_=src[:, half, :].unsqueeze(0).broadcast_to(
                    [64, NPAIR, CHUNK]))
        s["rhat"] = rhat

    def stage_avbc(i):
        b, c = chunks[i]
        s = st[i]
        e_sb, rhat = s["e"], s["rhat"]
        attnT_sb = atp.tile([128, NPAIR, CHUNK], F8, tag="attnT",
                            name=f"at{i}")
        av_t = {}

        def norm(j):
            nc.vector.tensor_mul(attnT_sb[:, j, :], av_t[j], rhat[:, j, :])

        for j in range(NPAIR):
            aps = attn.tile([128, CHUNK], F32, tag="attn", name=f"a{i}_{j}")
            nc.tensor.matmul(aps, vblk[:, b, j, :], e_sb[:, j, :],
                             start=True, stop=True)
            av_t[j] = aps
            if j >= 1:
                norm(j - 1)
        norm(NPAIR - 1)
        s["attnT"] = attnT_sb

    def stage_oproj(i):
        b, c = chunks[i]
        p0 = c * CHUNK
        s = st[i]
        attnT_sb, fbb = s["attnT"], s["fb"]
        o_sb = outp.tile([128, KQ, CHUNK], BF16, tag="osb", name=f"o{i}")
        for m in range(KQ):
            ops = pp.tile([128, CHUNK], F32, tag="pp", name=f"op{i}_{m}")
            for kc in range(KQ // 2):
                nc.tensor.matmul(ops,
                                 wo_sb[:, 2 * kc:2 * kc + 2,
                                       m * 128:(m + 1) * 128],
                                 attnT_sb[:, 2 * kc:2 * kc + 2, :],
                                 start=(kc == 0), stop=False,
                                 perf_mode=DRMODE)
            # residual: PSUM += I.T @ featb  (scaled feat+bout, bf16)
            nc.tensor.matmul(ops, id_sb[:], fbb[:, m, :],
                             start=False, stop=True)
            nc.scalar.copy(out=o_sb[:, m, :], in_=ops)
            if m == 2:
                nc.sync.dma_start(out=out_l[b, :, 0:3, p0:p0 + CHUNK],
                                  in_=o_sb[:, 0:3, :])
        nc.sync.dma_start(out=out_l[b, :, 3:KQ, p0:p0 + CHUNK],
                          in_=o_sb[:, 3:KQ, :])

    def scoped(fn, tag, i):
        with nc.named_scope(f"{tag}{i}"):
            fn(i)

    scoped(stage_qproj, "qp", 0)
    scoped(stage_qk, "qk", 0)
    n = len(chunks)
    for i in range(n):
        scoped(stage_recip, "rc", i)
        if i + 1 < n:
            scoped(stage_qproj, "qp", i + 1)
        scoped(stage_avbc, "av", i)
        if i + 1 < n:
            scoped(stage_qk, "qk", i + 1)
        scoped(stage_oproj, "op", i)


_NC_CACHE = {}


def _get_nc():
    if "nc" not in _NC_CACHE:
        _NC_CACHE["nc"] = build(debug=False)
    return _NC_CACHE["nc"]


def _prep_in_maps(feat, tokens, Wq, Wkv, Wout, bout):
    feat = np.ascontiguousarray(feat, dtype=np.float32).reshape(B, DQ, NQ)
    tokens = np.ascontiguousarray(tokens, dtype=np.float32)
    bout = np.asarray(bout, dtype=np.float32)
    shared = dict(
        wq8=np.ascontiguousarray(np.clip(
            (Wq * SW_Q).reshape(KQ, 128, DQ).transpose(1, 0, 2),
            -240, 240)).astype(NPF8),
        wk=np.ascontiguousarray(
            Wkv[:, :DQ].reshape(KKV, 128, DQ).transpose(1, 0, 2)).astype(NPBF),
        wv=np.ascontiguousarray(
            (Wkv[:, DQ:] * SW_V).reshape(KKV, 128, DQ).transpose(1, 0, 2)
        ).astype(NPBF),
        wo8=np.ascontiguousarray(np.clip(
            (Wout * SW_O).reshape(KQ, 128, DQ).transpose(1, 0, 2),
            -240, 240)).astype(NPF8),
        **_consts(),
    )
    bout_r = bout.reshape(KQ, 128).transpose(1, 0)[None, :, :, None]
    in_maps = []
    for cid in range(NCORES):
        sl = slice(BL * cid, BL * (cid + 1))
        fl = np.ascontiguousarray(
            feat[sl].reshape(BL, KQ, 128, NQ).transpose(0, 2, 1, 3))
        tl = np.ascontiguousarray(tokens[sl].reshape(BL * NT, DKV)).astype(NPBF)
        in_maps.append(dict(
            featb=((fl + bout_r) * SW_VO).astype(NPBF),
            feat8=np.clip(fl, -240, 240).astype(NPF8),
            tok_l=tl, **shared))
    return in_maps


def _install_ntff_hook():
    """The container's antenv lacks axon_hooks; register the NTFF profile
    hook from trn_agent_boot ourselves so trace=True yields HW exec times."""
    import types

    import antenv
    from trn_agent_boot.trn_boot import _ntff_profile_via_ctypes

    mod = types.ModuleType("antenv.axon_hooks")
    state = {"hook": None}
    mod.set_axon_ntff_profile_hook = lambda h: state.__setitem__("hook", h)
    mod.get_axon_ntff_profile_hook = lambda: state["hook"]
    sys.modules["antenv.axon_hooks"] = mod
    antenv.axon_hooks = mod
    mod.set_axon_ntff_profile_hook(
        _ntff_profile_via_ctypes("/opt/axon/libaxon_pjrt.so"))
    # the S3 artifact upload has no credentials here; make it a no-op
    import concourse.bass_utils as bu
    bu.upload_artifacts = lambda tmpdir: f"local:{tmpdir}"


def run(inputs, trace=False, trace_cores=None):
    nc = _get_nc()
    if trace:
        try:
            _install_ntff_hook()
        except Exception as e:  # profiling is best-effort
            print(f"ntff hook install failed: {e}", file=sys.stderr)
            trace = False
    in_maps = _prep_in_maps(**inputs)
    res = run_bass_kernel_spmd(nc, in_maps, core_ids=list(range(NCORES)),
                               trace=trace, trace_cores=trace_cores)
    outs = []
    for r in res.results:
        ol = r["out_l"].astype(np.float32) * (1.0 / SW_VO)
        outs.append(ol.transpose(0, 2, 1, 3).reshape(BL, DQ, T, HP, WP))
    return np.ascontiguousarray(np.concatenate(outs, axis=0)), res


def kernel(**inputs):
    return run(inputs, trace=False)[0]


# revision 20
# speedup vs baseline: 1.1595x; 1.1595x over previous
"""Trainium2 Bass kernel for nn_MultiHeadCrossAttention (B=16, Dq=768, H=12,
hd=64, Nq=1024, Nt=64, Dkv=384) with RoPE on q and k.

Sharding: pure data-parallel over batch, 2 batches per core across 8 cores.
No collectives.

v4 design (per-core, channel-major):
  qproj:  fp8e4 DoubleRow matmuls (Wq x64 host-scaled, feat in fp8): 2 K-tiles
          per instruction.
  q-RoPE: qc = qpsum * (cos*scale/64) [DVE -> bf16], qs = qc * tan
          [Pool, all-bf16 SBUF].
  k side: bf16 RoPE folded into kA/kB, written directly into block-diagonal
          tiles (kabblk) so each head-pair's scores take 2 K=128 matmuls.
  E = exp(scores) -> fp8 [ACT]. Denominators: DoubleRow over pair-pairs with
          an fp8 0/1 selection lhsT (padded to 16 cols for the 16B-aligned
          dual-fp8 ldweights rule). recip on DVE; bf16 r copy on ACT; PE
          broadcast via blhs matmul; bcs copy PSUM->SBUF bf16 on ACT.
  AV:     block-diagonal fp8 v (vblk): 1 matmul per pair.
  attnT = av * bcs -> fp8 [DVE] (normalize + fp8 convert fused)
  oproj:  fp8 DoubleRow (Wout x16, v x4); residual (feat+bout)*64 bf16 is
          accumulated into the same PSUM group via an identity matmul; PSUM ->
          bf16 out copies split ACT/DVE. Host divides by 64.
"""

import os
import sys
from contextlib import ExitStack

import numpy as np

sys.path.insert(0, "/opt/trn_rl_repo")

import concourse.bass as bass  # noqa: E402
import concourse.mybir as mybir  # noqa: E402
import concourse.tile as tile  # noqa: E402
from concourse import bacc  # noqa: E402
from concourse.bass_utils import run_bass_kernel_spmd  # noqa: E402

import ml_dtypes

F32 = mybir.dt.float32
BF16 = mybir.dt.bfloat16
F8 = mybir.dt.float8e4
NPBF = ml_dtypes.bfloat16
NPF8 = ml_dtypes.float8_e4m3
DRMODE = mybir.MatmulPerfMode.DoubleRow

B, DQ, T, HP, WP = 16, 768, 4, 16, 16
NQ = T * HP * WP            # 1024
NT, DKV = 64, 384
H, HD = 12, 64
SCALE = HD ** -0.5
NCORES = 8
BL = B // NCORES            # batches per core = 2
CHUNK = 512                 # query positions per chunk
NCH = NQ // CHUNK           # chunks per batch = 2
KQ = DQ // 128              # 6 contraction tiles for Dq
KKV = DKV // 128            # 3 contraction tiles for Dkv
NPAIR = H // 2              # 6 head pairs
SW_Q = 64.0                 # Wq host scale (fp8 range); undone in cq table
SW_V = 4.0                  # Wv host scale (attnT fp8 range)
SW_O = 16.0                 # Wout host scale (fp8 range)
SW_VO = SW_V * SW_O         # total out scale; undone on host


def _rope_tables(n):
    inv_freq = 1.0 / (10000.0 ** (np.arange(0, HD, 2, dtype=np.float64) / HD))
    freqs = np.arange(n, dtype=np.float64)[:, None] * inv_freq[None, :]
    emb = np.concatenate([freqs, freqs], axis=-1)  # [n, 64]
    return np.cos(emb).T, np.sin(emb).T            # [64, n] f64


def _consts():
    cq, sq = _rope_tables(NQ)          # [64, 1024] f64
    ck, sk = _rope_tables(NT)          # [64, 64]
    # q tables: RoPE scale and the 1/SW_Q unscale folded in; tan trick for qs
    cq2 = np.ascontiguousarray(
        np.tile(cq * SCALE / SW_Q, (2, 1))).astype(NPBF)       # [128, 1024]
    tanq2 = np.ascontiguousarray(np.tile(sq / cq, (2, 1))).astype(NPBF)
    # k tables: duplicated 2 heads (partitions), tiled (KQ, BL) along free to
    # match kT's [128, KQ, BL, 64] free layout
    ck2 = np.ascontiguousarray(np.tile(ck, (2, 2 * KQ))).astype(np.float32)
    sk2 = np.ascontiguousarray(np.tile(sk, (2, 2 * KQ))).astype(np.float32)
    eps = np.where(np.arange(HD) < HD // 2, -1.0, 1.0).astype(np.float32)
    epsv = np.ascontiguousarray(np.tile(eps, 2)[:, None])      # [128, 1]
    ident = np.eye(128, dtype='float32')
    # denominator lhsT: for pair j, col 2j sums partitions 0-63 (even head),
    # col 2j+1 sums partitions 64-127 (odd head). fp8 (0/1 exact). Padded to
    # 16 cols: dual-fp8 ldweights needs a 16B-aligned subtile stride.
    dlhs = np.zeros((128, NPAIR, 16), np.float32)
    for j in range(NPAIR):
        dlhs[:64, j, 2 * j] = 1.0
        dlhs[64:, j, 2 * j + 1] = 1.0
    # broadcast lhsT: for pair j, row 2j feeds cols 0-63, row 2j+1 cols 64-127
    blhs = np.zeros((H, NPAIR, 128), np.float32)
    for j in range(NPAIR):
        blhs[2 * j, j, :64] = 1.0
        blhs[2 * j + 1, j, 64:] = 1.0
    return dict(cq=cq2, tanq=tanq2, ck=ck2, sk=sk2, epsv=epsv, nepsv=-epsv,
                ident=ident.astype(NPBF), dlhs=dlhs.astype(NPF8),
                blhs=blhs.astype(NPBF))


def _sigma_dma(nc, out_ap, in_ap):
    """out = in with 32-partition halves swapped inside each 64 block."""
    for dst, src in ((0, 32), (32, 0), (64, 96), (96, 64)):
        nc.gpsimd.dma_start(out=out_ap[dst:dst + 32], in_=in_ap[src:src + 32])


def build(debug=False):
    nc = bacc.Bacc(None, target_bir_lowering=False, debug=debug)
    with tile.TileContext(nc) as tc:
        with tc.tile_pool(name="dram", bufs=1, space="DRAM") as dram:
            def din(name, shape, dt=F32):
                return dram.tile(shape, dt, kind="ExternalInput", name=name,
                                 uniquify=False)

            featb = din("featb", [BL, 128, KQ, NQ], BF16)
            feat8 = din("feat8", [BL, 128, KQ, NQ], F8)
            tok_l = din("tok_l", [BL * NT, DKV], BF16)
            wq8 = din("wq8", [128, KQ, DQ], F8)
            wk = din("wk", [128, KKV, DQ], BF16)
            wv = din("wv", [128, KKV, DQ], BF16)
            wo8 = din("wo8", [128, KQ, DQ], F8)
            cq = din("cq", [128, NQ], BF16)
            tanq = din("tanq", [128, NQ], BF16)
            ck = din("ck", [128, KQ * 128])
            sk = din("sk", [128, KQ * 128])
            epsv = din("epsv", [128, 1])
            nepsv = din("nepsv", [128, 1])
            ident = din("ident", [128, 128], BF16)
            dlhs = din("dlhs", [128, NPAIR, 16], F8)
            blhs = din("blhs", [H, NPAIR, 128], BF16)
            out_l = dram.tile([BL, 128, KQ, NQ], BF16, kind="ExternalOutput",
                              name="out_l", uniquify=False)

            with ExitStack() as body_ctx:
                global _body_ctx
                _body_ctx = body_ctx
                _body(nc, tc, featb, feat8, tok_l, wq8, wk, wv, wo8,
                      cq, tanq, ck, sk, epsv, nepsv, ident, dlhs, blhs,
                      out_l)
    nc.compile()
    return nc


def _body(nc, tc, featb, feat8, tok_l, wq8, wk, wv, wo8, cq, tanq, ck,
          sk, epsv, nepsv, ident, dlhs, blhs, out_l):
    MULT = mybir.AluOpType.mult
    ADD = mybir.AluOpType.add
    EXP = mybir.ActivationFunctionType.Exp

    ctx = _body_ctx
    consts = ctx.enter_context(tc.tile_pool(name="consts", bufs=1))
    kside = ctx.enter_context(tc.tile_pool(name="kside", bufs=1))
    ktmp = ctx.enter_context(tc.tile_pool(name="ktmp", bufs=1))
    featp = ctx.enter_context(tc.tile_pool(name="featp", bufs=2))
    qp = ctx.enter_context(tc.tile_pool(name="qp", bufs=2))
    ep = ctx.enter_context(tc.tile_pool(name="ep", bufs=2))
    atp = ctx.enter_context(tc.tile_pool(name="atp", bufs=2))
    outp = ctx.enter_context(tc.tile_pool(name="outp", bufs=2))
    rp = ctx.enter_context(tc.tile_pool(name="rp", bufs=2))

    pp = ctx.enter_context(tc.tile_pool(name="pp", bufs=3, space="PSUM"))
    attn = ctx.enter_context(tc.tile_pool(name="attn", bufs=4, space="PSUM"))
    dp = ctx.enter_context(tc.tile_pool(name="dp", bufs=1, space="PSUM"))

    # ---- load constants. sync ring carries the qproj/phase0 critical path.
    wq_sb = consts.tile([128, KQ, DQ], F8)
    nc.sync.dma_start(out=wq_sb, in_=wq8[:])
    tok_sb = consts.tile([128, DKV], BF16)
    nc.sync.dma_start(out=tok_sb, in_=tok_l[:])
    id_sb = consts.tile([128, 128], BF16)
    nc.sync.dma_start(out=id_sb, in_=ident[:])
    wk_sb = consts.tile([128, KKV, DQ], BF16)
    nc.sync.dma_start(out=wk_sb, in_=wk[:])
    cq_sb = consts.tile([128, NQ], BF16)
    nc.sync.dma_start(out=cq_sb, in_=cq[:])
    tanq_sb = consts.tile([128, NQ], BF16)
    nc.scalar.dma_start(out=tanq_sb, in_=tanq[:])
    wv_sb = consts.tile([128, KKV, DQ], BF16)
    nc.scalar.dma_start(out=wv_sb, in_=wv[:])
    ck_sb = consts.tile([128, KQ * 128], F32)
    nc.scalar.dma_start(out=ck_sb, in_=ck[:])
    sk_sb = consts.tile([128, KQ * 128], F32)
    nc.scalar.dma_start(out=sk_sb, in_=sk[:])
    eps_sb = consts.tile([128, 1], F32)
    nc.scalar.dma_start(out=eps_sb, in_=epsv[:])
    neps_sb = consts.tile([128, 1], F32)
    nc.scalar.dma_start(out=neps_sb, in_=nepsv[:])
    dlhs_sb = consts.tile([128, NPAIR, 16], F8)
    nc.scalar.dma_start(out=dlhs_sb, in_=dlhs[:])
    blhs_sb = consts.tile([H, NPAIR, 128], BF16)
    nc.scalar.dma_start(out=blhs_sb, in_=blhs[:])
    wo_sb = consts.tile([128, KQ, DQ], F8)
    nc.scalar.dma_start(out=wo_sb, in_=wo8[:])

    # ---- phase 0: tokensT, kT, k-RoPE into blockdiag kabblk, v/vblk ----
    _ph0 = nc.named_scope("ph0")
    _ph0.__enter__()
    tokT_sb = kside.tile([128, KKV, 128], BF16)
    for ct in range(KKV):
        tp = pp.tile([128, 128], BF16, tag="pp")
        nc.tensor.transpose(tp, tok_sb[:, ct * 128:(ct + 1) * 128], id_sb[:])
        nc.scalar.copy(out=tokT_sb[:, ct, :], in_=tp)

    # kT free layout: (pair, batch, token)
    kT_sb = kside.tile([128, KQ, BL, 64], F32)
    for m in range(KQ):
        kp = pp.tile([128, 128], F32, tag="pp")
        for kc in range(KKV):
            nc.tensor.matmul(kp, wk_sb[:, kc, m * 128:(m + 1) * 128],
                             tokT_sb[:, kc, :],
                             start=(kc == 0), stop=(kc == KKV - 1))
        nc.scalar.copy(out=kT_sb[:, m, :, :],
                       in_=kp.rearrange("p (b t) -> p b t", b=BL))

    # blockdiag kA/kB: kabblk[p, j, b, 0/1, :]; even head top-left 64x64,
    # odd head bottom-right. Off-diagonal quarters stay zero.
    kabblk = kside.tile([128, KQ, BL, 2, 128], BF16)
    nc.scalar.memzero(kabblk[:])
    t1 = ktmp.tile([128, KQ, BL, 64], F32, tag="t1")
    t2 = ktmp.tile([128, KQ, BL, 64], F32, tag="t2")
    t1s = ktmp.tile([128, KQ, BL, 64], F32, tag="t1s")
    t2s = ktmp.tile([128, KQ, BL, 64], F32, tag="t2s")
    nc.gpsimd.tensor_mul(t1, kT_sb[:], ck_sb[:].rearrange(
        "p (j b t) -> p j b t", j=KQ, b=BL))
    nc.gpsimd.tensor_mul(t2, kT_sb[:], sk_sb[:].rearrange(
        "p (j b t) -> p j b t", j=KQ, b=BL))
    _sigma_dma(nc, t1s, t1)
    _sigma_dma(nc, t2s, t2)
    # kA = k_rot = t1 + eps*sigma(t2);  kB = t2 - eps*sigma(t1)
    nc.vector.scalar_tensor_tensor(
        out=kabblk[0:64, :, :, 0, 0:64], in0=t2s[0:64], scalar=eps_sb[0:64],
        in1=t1[0:64], op0=MULT, op1=ADD)
    nc.vector.scalar_tensor_tensor(
        out=kabblk[64:128, :, :, 0, 64:128], in0=t2s[64:128],
        scalar=eps_sb[64:128], in1=t1[64:128], op0=MULT, op1=ADD)
    nc.vector.scalar_tensor_tensor(
        out=kabblk[0:64, :, :, 1, 0:64], in0=t1s[0:64], scalar=neps_sb[0:64],
        in1=t2[0:64], op0=MULT, op1=ADD)
    nc.vector.scalar_tensor_tensor(
        out=kabblk[64:128, :, :, 1, 64:128], in0=t1s[64:128],
        scalar=neps_sb[64:128], in1=t2[64:128], op0=MULT, op1=ADD)

    # v, natural [token, dim] layout, duplicated across partition halves,
    # viewed as [128, b, pair, half, 64]
    vv_sb = kside.tile([128, BL, NPAIR, 2, 64], BF16)
    for b in range(BL):
        for nn in range(2):   # 2 chunks of 3 pairs = 384 dims
            vps = pp.tile([128, 384], F32, tag="pp")
            for half in range(2):
                for kc in range(KKV):
                    nc.tensor.matmul(
                        vps[64 * half:64 * half + 64, :],
                        tokT_sb[:, kc, b * 64:(b + 1) * 64],
                        wv_sb[:, kc, nn * 384:(nn + 1) * 384],
                        start=(kc == 0), stop=(kc == KKV - 1))
            nc.scalar.copy(
                out=vv_sb[:, b, 3 * nn:3 * nn + 3, :, :],
                in_=vps.rearrange("p (j h t) -> p j h t", j=3, h=2))

    vblk = kside.tile([128, BL, NPAIR, 128], F8)
    nc.scalar.memzero(vblk[:])
    nc.scalar.copy(out=vblk[0:64, :, :, 0:64], in_=vv_sb[0:64, :, :, 0, :])
    nc.scalar.copy(out=vblk[64:128, :, :, 64:128],
                   in_=vv_sb[64:128, :, :, 1, :])

    _ph0.__exit__(None, None, None)

    # ---- main loop: software-pipelined across the 4 (batch, chunk) steps ----
    chunks = [(b, c) for b in range(BL) for c in range(NCH)]
    st = {}

    def stage_qproj(i):
        b, c = chunks[i]
        p0 = c * CHUNK
        f8b = featp.tile([128, KQ, CHUNK], F8, tag="f8", name=f"f8_{i}")
        nc.sync.dma_start(out=f8b, in_=feat8[b, :, :, p0:p0 + CHUNK])
        fbb = featp.tile([128, KQ, CHUNK], BF16, tag="fb", name=f"fb{i}")
        nc.sync.dma_start(out=fbb, in_=featb[b, :, :, p0:p0 + CHUNK])
        qc_sb = qp.tile([128, KQ, CHUNK], BF16, tag="qc", name=f"qc{i}")
        qs_sb = qp.tile([128, KQ, CHUNK], BF16, tag="qs", name=f"qs{i}")
        for m in range(KQ):
            qps = pp.tile([128, CHUNK], F32, tag="pp", name=f"qp{i}_{m}")
            for kc in range(KQ // 2):
                nc.tensor.matmul(qps,
                                 wq_sb[:, 2 * kc:2 * kc + 2,
                                       m * 128:(m + 1) * 128],
                                 f8b[:, 2 * kc:2 * kc + 2, :],
                                 start=(kc == 0), stop=(kc == KQ // 2 - 1),
                                 perf_mode=DRMODE)
            nc.vector.tensor_mul(qc_sb[:, m, :], qps, cq_sb[:, p0:p0 + CHUNK])
            nc.gpsimd.tensor_mul(qs_sb[:, m, :], qc_sb[:, m, :],
                                 tanq_sb[:, p0:p0 + CHUNK])
        st[i] = dict(qc=qc_sb, qs=qs_sb, fb=fbb)

    def stage_qk(i):
        b, c = chunks[i]
        s = st[i]
        qc_sb, qs_sb = s["qc"], s["qs"]
        e_sb = ep.tile([128, NPAIR, CHUNK], F8, tag="e", name=f"e{i}")
        dps = dp.tile([16, CHUNK], F32, tag="den", name=f"d{i}")

        def qk1(j):
            sps = attn.tile([128, CHUNK], F32, tag="attn", name=f"s{i}_{j}")
            nc.tensor.matmul(sps, kabblk[:, j, b, 0, :], qc_sb[:, j, :],
                             start=True, stop=False)
            nc.tensor.matmul(sps, kabblk[:, j, b, 1, :], qs_sb[:, j, :],
                             start=False, stop=True)
            nc.scalar.activation(out=e_sb[:, j, :], in_=sps, func=EXP)

        def denom(t):
            nc.tensor.matmul(dps, dlhs_sb[:, 2 * t:2 * t + 2, :],
                             e_sb[:, 2 * t:2 * t + 2, :],
                             start=(t == 0), stop=(t == NPAIR // 2 - 1),
                             perf_mode=DRMODE)

        for j in range(NPAIR):
            qk1(j)
            if j % 2 == 1 and j >= 3:
                denom(j // 2 - 1)
        denom(NPAIR // 2 - 1)
        s["e"], s["dps"] = e_sb, dps

    def stage_recip(i):
        s = st[i]
        r32 = rp.tile([H, CHUNK], F32, tag="r32", name=f"r32_{i}")
        nc.vector.reciprocal_approx_fast(out=r32, in_=s["dps"][0:H])
        r_sb = rp.tile([H, CHUNK], BF16, tag="r", name=f"r{i}")
        nc.scalar.copy(out=r_sb, in_=r32)
        s["r"] = r_sb

    def stage_avbc(i):
        b, c = chunks[i]
        s = st[i]
        e_sb, r_sb = s["e"], s["r"]
        attnT_sb = atp.tile([128, NPAIR, CHUNK], F8, tag="attnT",
                            name=f"at{i}")

        def av(j):
            aps = attn.tile([128, CHUNK], F32, tag="attn", name=f"a{i}_{j}")
            nc.tensor.matmul(aps, vblk[:, b, j, :], e_sb[:, j, :],
                             start=True, stop=True)
            return aps

        def bcast(j):
            bps = attn.tile([128, CHUNK], F32, tag="attn", name=f"b{i}_{j}")
            nc.tensor.matmul(bps, blhs_sb[:, j, :],
                             r_sb[:], start=True, stop=True)
            # stage to SBUF (DVE may read only one PSUM operand)
            bcs = rp.tile([128, CHUNK], BF16, tag="bcs", bufs=3,
                          name=f"bc{i}_{j}")
            nc.scalar.copy(out=bcs, in_=bps)
            return bcs

        av_t, bc_t = {}, {}

        def norm(j):
            nc.vector.tensor_mul(attnT_sb[:, j, :], av_t[j], bc_t[j])

        for j in range(NPAIR):
            av_t[j] = av(j)
            bc_t[j] = bcast(j)
            if j >= 1:
                norm(j - 1)
        norm(NPAIR - 1)
        s["attnT"] = attnT_sb

    def stage_oproj(i):
        b, c = chunks[i]
        p0 = c * CHUNK
        s = st[i]
        attnT_sb, fbb = s["attnT"], s["fb"]
        o_sb = outp.tile([128, KQ, CHUNK], BF16, tag="osb", name=f"o{i}")
        for m in range(KQ):
            ops = pp.tile([128, CHUNK], F32, tag="pp", name=f"op{i}_{m}")
            for kc in range(KQ // 2):
                nc.tensor.matmul(ops,
                                 wo_sb[:, 2 * kc:2 * kc + 2,
                                       m * 128:(m + 1) * 128],
                                 attnT_sb[:, 2 * kc:2 * kc + 2, :],
                                 start=(kc == 0), stop=False,
                                 perf_mode=DRMODE)
            # residual: PSUM += I.T @ featb  (scaled feat+bout, bf16)
            nc.tensor.matmul(ops, id_sb[:], fbb[:, m, :],
                             start=False, stop=True)
            if m % 2 == 0:
                nc.scalar.copy(out=o_sb[:, m, :], in_=ops)
            else:
                nc.vector.tensor_scalar_add(o_sb[:, m, :], ops, 0.0)
            if m == 2:
                nc.sync.dma_start(out=out_l[b, :, 0:3, p0:p0 + CHUNK],
                                  in_=o_sb[:, 0:3, :])
        nc.sync.dma_start(out=out_l[b, :, 3:KQ, p0:p0 + CHUNK],
                          in_=o_sb[:, 3:KQ, :])

    def scoped(fn, tag, i):
        with nc.named_scope(f"{tag}{i}"):
            fn(i)

    scoped(stage_qproj, "qp", 0)
    scoped(stage_qk, "qk", 0)
    n = len(chunks)
    for i in range(n):
        scoped(stage_recip, "rc", i)
        if i + 1 < n:
            scoped(stage_qproj, "qp", i + 1)
        scoped(stage_avbc, "av", i)
        if i + 1 < n:
            scoped(stage_qk, "qk", i + 1)
        scoped(stage_oproj, "op", i)


_NC_CACHE = {}


def _get_nc():
    if "nc" not in _NC_CACHE:
        _NC_CACHE["nc"] = build(debug=False)
    return _NC_CACHE["nc"]


def _prep_in_maps(feat, tokens, Wq, Wkv, Wout, bout):
    feat = np.ascontiguousarray(feat, dtype=np.float32).reshape(B, DQ, NQ)
    tokens = np.ascontiguousarray(tokens, dtype=np.float32)
    bout = np.asarray(bout, dtype=np.float32)
    shared = dict(
        wq8=np.ascontiguousarray(np.clip(
            (Wq * SW_Q).reshape(KQ, 128, DQ).transpose(1, 0, 2),
            -240, 240)).astype(NPF8),
        wk=np.ascontiguousarray(
            Wkv[:, :DQ].reshape(KKV, 128, DQ).transpose(1, 0, 2)).astype(NPBF),
        wv=np.ascontiguousarray(
            (Wkv[:, DQ:] * SW_V).reshape(KKV, 128, DQ).transpose(1, 0, 2)
        ).astype(NPBF),
        wo8=np.ascontiguousarray(np.clip(
            (Wout * SW_O).reshape(KQ, 128, DQ).transpose(1, 0, 2),
            -240, 240)).astype(NPF8),
        **_consts(),
    )
    bout_r = bout.reshape(KQ, 128).transpose(1, 0)[None, :, :, None]
    in_maps = []
    for cid in range(NCORES):
        sl = slice(BL * cid, BL * (cid + 1))
        fl = np.ascontiguousarray(
            feat[sl].reshape(BL, KQ, 128, NQ).transpose(0, 2, 1, 3))
        tl = np.ascontiguousarray(tokens[sl].reshape(BL * NT, DKV)).astype(NPBF)
        in_maps.append(dict(
            featb=((fl + bout_r) * SW_VO).astype(NPBF),
            feat8=np.clip(fl, -240, 240).astype(NPF8),
            tok_l=tl, **shared))
    return in_maps


def _install_ntff_hook():
    """The container's antenv lacks axon_hooks; register the NTFF profile
    hook from trn_agent_boot ourselves so trace=True yields HW exec times."""
    import types

    import antenv
    from trn_agent_boot.trn_boot import _ntff_profile_via_ctypes

    mod = types.ModuleType("antenv.axon_hooks")
    state = {"hook": None}
    mod.set_axon_ntff_profile_hook = lambda h: state.__setitem__("hook", h)
    mod.get_axon_ntff_profile_hook = lambda: state["hook"]
    sys.modules["antenv.axon_hooks"] = mod
    antenv.axon_hooks = mod
    mod.set_axon_ntff_profile_hook(
        _ntff_profile_via_ctypes("/opt/axon/libaxon_pjrt.so"))
    # the S3 artifact upload has no credentials here; make it a no-op
    import concourse.bass_utils as bu
    bu.upload_artifacts = lambda tmpdir: f"local:{tmpdir}"


def run(inputs, trace=False, trace_cores=None):
    nc = _get_nc()
    if trace:
        try:
            _install_ntff_hook()
        except Exception as e:  # profiling is best-effort
            print(f"ntff hook install failed: {e}", file=sys.stderr)
            trace = False
    in_maps = _prep_in_maps(**inputs)
    res = run_bass_kernel_spmd(nc, in_maps, core_ids=list(range(NCORES)),
                               trace=trace, trace_cores=trace_cores)
    outs = []
    for r in res.results:
        ol = r["out_l"].astype(np.float32) * (1.0 / SW_VO)
        outs.append(ol.transpose(0, 2, 1, 3).reshape(BL, DQ, T, HP, WP))
    return np.ascontiguousarray(np.concatenate(outs, axis=0)), res


def kernel(**inputs):
    return run(inputs, trace=False)[0]


# revision 21
# speedup vs baseline: 1.2978x; 1.1193x over previous
"""Trainium2 Bass kernel for nn_MultiHeadCrossAttention (B=16, Dq=768, H=12,
hd=64, Nq=1024, Nt=64, Dkv=384) with RoPE on q and k.

Sharding: pure data-parallel over batch, 2 batches per core across 8 cores.
No collectives.

v5 design (per-core, channel-major):
  qproj:  fp8e4 DoubleRow matmuls (Wq x64 host-scaled, feat in fp8): 2 K-tiles
          per instruction.
  q-RoPE: qc = qpsum * (cos*scale/64) [DVE -> fp8], qs = qc * tan [Pool, fp8].
          qc/qs interleaved = the 2 K-subtiles of the DoubleRow scores matmul.
  k side: RoPE via a second host-permuted Wk (wks = rotate-half rows) so
          kT_sigma comes from the PE instead of partition-swap DMAs:
            kA = kT*ck + kTs*(eps*sk),  kB = kT*sk - kTs*(eps*ck)
          written flat then packed into block-diagonal fp8 kabblk (even head
          top-left 64x64, odd head bottom-right): ONE DoubleRow matmul per
          head-pair computes kA.T@qc + kB.T@qs for both heads.
  E = exp(scores) -> fp8 [ACT]. Denominators: DoubleRow over pair-pairs with
          an fp8 0/1 selection lhsT (padded to 16 cols for the 16B-aligned
          dual-fp8 ldweights rule). recip on DVE -> bf16 copy on ACT.
  r broadcast: DRAM round-trip; 12 stride-0 reads (one per head) spread
          across DMA queues. No PE broadcast, no PSUM->SBUF staging copies.
  AV:     block-diagonal fp8 v (vblk): 1 matmul per pair.
  attnT = av * rhat -> fp8 [DVE] (normalize + fp8 convert fused)
  oproj:  fp8 DoubleRow (Wout x16, v x4); residual (feat+bout)*64 bf16 is
          accumulated into the same PSUM group via an identity matmul; PSUM ->
          bf16 out copies on ACT. Host divides by 64.
"""

import os
import sys
from contextlib import ExitStack

import numpy as np

sys.path.insert(0, "/opt/trn_rl_repo")

import concourse.bass as bass  # noqa: E402
import concourse.mybir as mybir  # noqa: E402
import concourse.tile as tile  # noqa: E402
from concourse import bacc  # noqa: E402
from concourse.bass_utils import run_bass_kernel_spmd  # noqa: E402

import ml_dtypes

F32 = mybir.dt.float32
BF16 = mybir.dt.bfloat16
F8 = mybir.dt.float8e4
NPBF = ml_dtypes.bfloat16
NPF8 = ml_dtypes.float8_e4m3
DRMODE = mybir.MatmulPerfMode.DoubleRow

B, DQ, T, HP, WP = 16, 768, 4, 16, 16
NQ = T * HP * WP            # 1024
NT, DKV = 64, 384
H, HD = 12, 64
SCALE = HD ** -0.5
NCORES = 8
BL = B // NCORES            # batches per core = 2
CHUNK = 512                 # query positions per chunk
NCH = NQ // CHUNK           # chunks per batch = 2
KQ = DQ // 128              # 6 contraction tiles for Dq
KKV = DKV // 128            # 3 contraction tiles for Dkv
NPAIR = H // 2              # 6 head pairs
SW_Q = 64.0                 # Wq host scale (fp8 range); undone in cq table
SW_V = 4.0                  # Wv host scale (attnT fp8 range)
SW_O = 16.0                 # Wout host scale (fp8 range)
SW_VO = SW_V * SW_O         # total out scale; undone on host


def _rope_tables(n):
    inv_freq = 1.0 / (10000.0 ** (np.arange(0, HD, 2, dtype=np.float64) / HD))
    freqs = np.arange(n, dtype=np.float64)[:, None] * inv_freq[None, :]
    emb = np.concatenate([freqs, freqs], axis=-1)  # [n, 64]
    return np.cos(emb).T, np.sin(emb).T            # [64, n] f64


def _consts():
    cq, sq = _rope_tables(NQ)          # [64, 1024] f64
    ck, sk = _rope_tables(NT)          # [64, 64]
    # q tables: RoPE scale and the 1/SW_Q unscale folded in; tan trick for qs
    cq2 = np.ascontiguousarray(
        np.tile(cq * SCALE / SW_Q, (2, 1))).astype(NPBF)       # [128, 1024]
    tanq2 = np.ascontiguousarray(np.tile(sq / cq, (2, 1))).astype(NPBF)
    # k tables, free layout (pair, batch, token); eps folded into the
    # sigma-term tables
    eps = np.where(np.arange(HD) < HD // 2, -1.0, 1.0)[:, None]  # [64, 1]
    ck2 = np.ascontiguousarray(np.tile(ck, (2, 2 * KQ))).astype(np.float32)
    sk2 = np.ascontiguousarray(np.tile(sk, (2, 2 * KQ))).astype(np.float32)
    skE = np.ascontiguousarray(np.tile(eps * sk, (2, 2 * KQ))).astype(
        np.float32)
    ckE = np.ascontiguousarray(np.tile(-eps * ck, (2, 2 * KQ))).astype(
        np.float32)
    ident = np.eye(128, dtype='float32')
    # denominator lhsT: for pair j, col 2j sums partitions 0-63 (even head),
    # col 2j+1 sums partitions 64-127 (odd head). fp8 (0/1 exact). Padded to
    # 16 cols: dual-fp8 ldweights needs a 16B-aligned subtile stride.
    dlhs = np.zeros((128, NPAIR, 16), np.float32)
    for j in range(NPAIR):
        dlhs[:64, j, 2 * j] = 1.0
        dlhs[64:, j, 2 * j + 1] = 1.0
    return dict(cq=cq2, tanq=tanq2, ck=ck2, sk=sk2, skE=skE, ckE=ckE,
                ident=ident.astype(NPBF), dlhs=dlhs.astype(NPF8))


def build(debug=False):
    nc = bacc.Bacc(None, target_bir_lowering=False, debug=debug)
    with tile.TileContext(nc) as tc:
        with tc.tile_pool(name="dram", bufs=1, space="DRAM") as dram:
            def din(name, shape, dt=F32):
                return dram.tile(shape, dt, kind="ExternalInput", name=name,
                                 uniquify=False)

            featb = din("featb", [BL, 128, KQ, NQ], BF16)
            feat8 = din("feat8", [BL, 128, KQ, NQ], F8)
            tok_l = din("tok_l", [BL * NT, DKV], BF16)
            wq8 = din("wq8", [128, KQ, DQ], F8)
            wk = din("wk", [128, KKV, DQ], BF16)
            wks = din("wks", [128, KKV, DQ], BF16)
            wv = din("wv", [128, KKV, DQ], BF16)
            wo8 = din("wo8", [128, KQ, DQ], F8)
            cq = din("cq", [128, NQ], BF16)
            tanq = din("tanq", [128, NQ], BF16)
            ck = din("ck", [128, KQ * 128])
            sk = din("sk", [128, KQ * 128])
            skE = din("skE", [128, KQ * 128])
            ckE = din("ckE", [128, KQ * 128])
            ident = din("ident", [128, 128], BF16)
            dlhs = din("dlhs", [128, NPAIR, 16], F8)
            r_dram = dram.tile([2, H, CHUNK], BF16, name="r_scratch")
            out_l = dram.tile([BL, 128, KQ, NQ], BF16, kind="ExternalOutput",
                              name="out_l", uniquify=False)

            with ExitStack() as body_ctx:
                global _body_ctx
                _body_ctx = body_ctx
                _body(nc, tc, featb, feat8, tok_l, wq8, wk, wks, wv, wo8,
                      cq, tanq, ck, sk, skE, ckE, ident, dlhs, r_dram,
                      out_l)
    nc.compile()
    return nc


def _body(nc, tc, featb, feat8, tok_l, wq8, wk, wks, wv, wo8, cq, tanq, ck,
          sk, skE, ckE, ident, dlhs, r_dram, out_l):
    EXP = mybir.ActivationFunctionType.Exp

    ctx = _body_ctx
    consts = ctx.enter_context(tc.tile_pool(name="consts", bufs=1))
    kside = ctx.enter_context(tc.tile_pool(name="kside", bufs=1))
    ktmp = ctx.enter_context(tc.tile_pool(name="ktmp", bufs=1))
    featp = ctx.enter_context(tc.tile_pool(name="featp", bufs=2))
    qp = ctx.enter_context(tc.tile_pool(name="qp", bufs=2))
    ep = ctx.enter_context(tc.tile_pool(name="ep", bufs=2))
    atp = ctx.enter_context(tc.tile_pool(name="atp", bufs=2))
    outp = ctx.enter_context(tc.tile_pool(name="outp", bufs=2))
    rp = ctx.enter_context(tc.tile_pool(name="rp", bufs=2))

    pp = ctx.enter_context(tc.tile_pool(name="pp", bufs=3, space="PSUM"))
    attn = ctx.enter_context(tc.tile_pool(name="attn", bufs=4, space="PSUM"))
    dp = ctx.enter_context(tc.tile_pool(name="dp", bufs=1, space="PSUM"))

    # ---- load constants. sync ring order = phase0/qproj critical path.
    tok_sb = consts.tile([128, DKV], BF16)
    nc.sync.dma_start(out=tok_sb, in_=tok_l[:])
    id_sb = consts.tile([128, 128], BF16)
    nc.sync.dma_start(out=id_sb, in_=ident[:])
    wk_sb = consts.tile([128, KKV, DQ], BF16)
    nc.sync.dma_start(out=wk_sb, in_=wk[:])
    wks_sb = consts.tile([128, KKV, DQ], BF16)
    nc.sync.dma_start(out=wks_sb, in_=wks[:])
    wq_sb = consts.tile([128, KQ, DQ], F8)
    nc.sync.dma_start(out=wq_sb, in_=wq8[:])
    cq_sb = consts.tile([128, NQ], BF16)
    nc.sync.dma_start(out=cq_sb, in_=cq[:])
    wv_sb = consts.tile([128, KKV, DQ], BF16)
    nc.scalar.dma_start(out=wv_sb, in_=wv[:])
    tanq_sb = consts.tile([128, NQ], BF16)
    nc.scalar.dma_start(out=tanq_sb, in_=tanq[:])
    ck_sb = consts.tile([128, KQ * 128], F32)
    nc.scalar.dma_start(out=ck_sb, in_=ck[:])
    sk_sb = consts.tile([128, KQ * 128], F32)
    nc.scalar.dma_start(out=sk_sb, in_=sk[:])
    skE_sb = consts.tile([128, KQ * 128], F32)
    nc.scalar.dma_start(out=skE_sb, in_=skE[:])
    ckE_sb = consts.tile([128, KQ * 128], F32)
    nc.scalar.dma_start(out=ckE_sb, in_=ckE[:])
    dlhs_sb = consts.tile([128, NPAIR, 16], F8)
    nc.scalar.dma_start(out=dlhs_sb, in_=dlhs[:])
    wo_sb = consts.tile([128, KQ, DQ], F8)
    nc.scalar.dma_start(out=wo_sb, in_=wo8[:])

    # ---- phase 0: tokensT, kT/kTs, k-RoPE into blockdiag kabblk, v/vblk ----
    _ph0 = nc.named_scope("ph0")
    _ph0.__enter__()
    tokT_sb = kside.tile([128, KKV, 128], BF16)
    for ct in range(KKV):
        tp = pp.tile([128, 128], BF16, tag="pp")
        nc.tensor.transpose(tp, tok_sb[:, ct * 128:(ct + 1) * 128], id_sb[:])
        nc.scalar.copy(out=tokT_sb[:, ct, :], in_=tp)

    # kT / kT_sigma, free layout (pair, batch, token)
    kT_sb = kside.tile([128, KQ, BL, 64], F32)
    kTs_sb = kside.tile([128, KQ, BL, 64], F32)
    for dst_sb, w_sb in ((kT_sb, wk_sb), (kTs_sb, wks_sb)):
        for m in range(KQ):
            kp = pp.tile([128, 128], F32, tag="pp")
            for kc in range(KKV):
                nc.tensor.matmul(kp, w_sb[:, kc, m * 128:(m + 1) * 128],
                                 tokT_sb[:, kc, :],
                                 start=(kc == 0), stop=(kc == KKV - 1))
            nc.scalar.copy(out=dst_sb[:, m, :, :],
                           in_=kp.rearrange("p (b t) -> p b t", b=BL))

    # kA = kT*ck + kTs*skE ; kB = kT*sk + kTs*ckE   (eps folded in tables)
    ckv = ck_sb[:].rearrange("p (j b t) -> p j b t", j=KQ, b=BL)
    skv = sk_sb[:].rearrange("p (j b t) -> p j b t", j=KQ, b=BL)
    skEv = skE_sb[:].rearrange("p (j b t) -> p j b t", j=KQ, b=BL)
    ckEv = ckE_sb[:].rearrange("p (j b t) -> p j b t", j=KQ, b=BL)
    t1 = ktmp.tile([128, KQ, BL, 64], F32, tag="t1")
    t2 = ktmp.tile([128, KQ, BL, 64], F32, tag="t2")
    m1 = ktmp.tile([128, KQ, BL, 64], F32, tag="m1")
    m2 = ktmp.tile([128, KQ, BL, 64], F32, tag="m2")
    nc.vector.tensor_mul(t1, kT_sb[:], ckv)
    nc.vector.tensor_mul(m1, kTs_sb[:], skEv)
    nc.vector.tensor_mul(t2, kT_sb[:], skv)
    nc.vector.tensor_mul(m2, kTs_sb[:], ckEv)
    kA_sb = ktmp.tile([128, KQ, BL, 64], F8, tag="kA")
    kB_sb = ktmp.tile([128, KQ, BL, 64], F8, tag="kB")
    nc.vector.tensor_add(kA_sb, t1, m1)
    nc.vector.tensor_add(kB_sb, t2, m2)

    # blockdiag pack: kabblk[p, j, b, 0/1, :]; even head top-left 64x64,
    # odd head bottom-right; off-diagonal quarters zero.
    kabblk = kside.tile([128, KQ, BL, 2, 128], F8)
    nc.scalar.memzero(kabblk[:])
    nc.scalar.copy(out=kabblk[0:64, :, :, 0, 0:64], in_=kA_sb[0:64])
    nc.scalar.copy(out=kabblk[64:128, :, :, 0, 64:128], in_=kA_sb[64:128])
    nc.scalar.copy(out=kabblk[0:64, :, :, 1, 0:64], in_=kB_sb[0:64])
    nc.scalar.copy(out=kabblk[64:128, :, :, 1, 64:128], in_=kB_sb[64:128])

    # v, natural [token, dim] layout, duplicated across partition halves,
    # viewed as [128, b, pair, half, 64]
    vv_sb = kside.tile([128, BL, NPAIR, 2, 64], BF16)
    for b in range(BL):
        for nn in range(2):   # 2 chunks of 3 pairs = 384 dims
            vps = pp.tile([128, 384], F32, tag="pp")
            for half in range(2):
                for kc in range(KKV):
                    nc.tensor.matmul(
                        vps[64 * half:64 * half + 64, :],
                        tokT_sb[:, kc, b * 64:(b + 1) * 64],
                        wv_sb[:, kc, nn * 384:(nn + 1) * 384],
                        start=(kc == 0), stop=(kc == KKV - 1))
            nc.scalar.copy(
                out=vv_sb[:, b, 3 * nn:3 * nn + 3, :, :],
                in_=vps.rearrange("p (j h t) -> p j h t", j=3, h=2))

    vblk = kside.tile([128, BL, NPAIR, 128], F8)
    nc.scalar.memzero(vblk[:])
    nc.scalar.copy(out=vblk[0:64, :, :, 0:64], in_=vv_sb[0:64, :, :, 0, :])
    nc.scalar.copy(out=vblk[64:128, :, :, 64:128],
                   in_=vv_sb[64:128, :, :, 1, :])

    _ph0.__exit__(None, None, None)

    # ---- main loop: software-pipelined across the 4 (batch, chunk) steps ----
    chunks = [(b, c) for b in range(BL) for c in range(NCH)]
    st = {}

    def stage_qproj(i):
        b, c = chunks[i]
        p0 = c * CHUNK
        f8b = featp.tile([128, KQ, CHUNK], F8, tag="f8", name=f"f8_{i}")
        nc.sync.dma_start(out=f8b, in_=feat8[b, :, :, p0:p0 + CHUNK])
        fbb = featp.tile([128, KQ, CHUNK], BF16, tag="fb", name=f"fb{i}")
        nc.sync.dma_start(out=fbb, in_=featb[b, :, :, p0:p0 + CHUNK])
        # qc/qs interleaved: the 2 K-subtiles of the DoubleRow scores matmul
        qcs = qp.tile([128, KQ, 2, CHUNK], F8, tag="qcs", name=f"qcs{i}")
        for m in range(KQ):
            qps = pp.tile([128, CHUNK], F32, tag="pp", name=f"qp{i}_{m}")
            for kc in range(KQ // 2):
                nc.tensor.matmul(qps,
                                 wq_sb[:, 2 * kc:2 * kc + 2,
                                       m * 128:(m + 1) * 128],
                                 f8b[:, 2 * kc:2 * kc + 2, :],
                                 start=(kc == 0), stop=(kc == KQ // 2 - 1),
                                 perf_mode=DRMODE)
            nc.vector.tensor_mul(qcs[:, m, 0, :], qps, cq_sb[:, p0:p0 + CHUNK])
            nc.gpsimd.tensor_mul(qcs[:, m, 1, :], qcs[:, m, 0, :],
                                 tanq_sb[:, p0:p0 + CHUNK])
        st[i] = dict(qcs=qcs, fb=fbb)

    def stage_qk(i):
        b, c = chunks[i]
        s = st[i]
        qcs = s["qcs"]
        e_sb = ep.tile([128, NPAIR, CHUNK], F8, tag="e", name=f"e{i}")
        dps = dp.tile([16, CHUNK], F32, tag="den", name=f"d{i}")

        def qk1(j):
            sps = attn.tile([128, CHUNK], F32, tag="attn", name=f"s{i}_{j}")
            nc.tensor.matmul(sps, kabblk[:, j, b, :, :], qcs[:, j, :, :],
                             start=True, stop=True, perf_mode=DRMODE)
            nc.scalar.activation(out=e_sb[:, j, :], in_=sps, func=EXP)

        def denom(t):
            nc.tensor.matmul(dps, dlhs_sb[:, 2 * t:2 * t + 2, :],
                             e_sb[:, 2 * t:2 * t + 2, :],
                             start=(t == 0), stop=(t == NPAIR // 2 - 1),
                             perf_mode=DRMODE)

        for j in range(NPAIR):
            qk1(j)
            if j % 2 == 1 and j >= 3:
                denom(j // 2 - 1)
        denom(NPAIR // 2 - 1)
        s["e"], s["dps"] = e_sb, dps

    def stage_recip(i):
        s = st[i]
        r32 = rp.tile([H, CHUNK], F32, tag="r32", name=f"r32_{i}")
        nc.vector.reciprocal_approx_fast(out=r32, in_=s["dps"][0:H])
        r_sb = rp.tile([H, CHUNK], BF16, tag="r", name=f"r{i}")
        nc.scalar.copy(out=r_sb, in_=r32)
        # broadcast r to 128 partitions via DRAM round-trip (stride-0 reads,
        # one dma_start per head to spread across DMA queues):
        # rhat[p, j, q] = r[2j + (p>=64), q]
        slot = i % 2
        nc.sync.dma_start(out=r_dram[slot], in_=r_sb)
        rhat = rp.tile([128, NPAIR, CHUNK], BF16, tag="rhat", name=f"rh{i}")
        for j in range(NPAIR):
            for half in range(2):
                nc.sync.dma_start(
                    out=rhat[64 * half:64 * half + 64, j, :],
                    in_=r_dram[slot, 2 * j + half, :].unsqueeze(0)
                    .broadcast_to([64, CHUNK]))
        s["rhat"] = rhat

    def stage_avbc(i):
        b, c = chunks[i]
        s = st[i]
        e_sb, rhat = s["e"], s["rhat"]
        attnT_sb = atp.tile([128, NPAIR, CHUNK], F8, tag="attnT",
                            name=f"at{i}")
        av_t = {}

        def norm(j):
            nc.vector.tensor_mul(attnT_sb[:, j, :], av_t[j], rhat[:, j, :])

        for j in range(NPAIR):
            aps = attn.tile([128, CHUNK], F32, tag="attn", name=f"a{i}_{j}")
            nc.tensor.matmul(aps, vblk[:, b, j, :], e_sb[:, j, :],
                             start=True, stop=True)
            av_t[j] = aps
            if j >= 1:
                norm(j - 1)
        norm(NPAIR - 1)
        s["attnT"] = attnT_sb

    def stage_oproj(i):
        b, c = chunks[i]
        p0 = c * CHUNK
        s = st[i]
        attnT_sb, fbb = s["attnT"], s["fb"]
        o_sb = outp.tile([128, KQ, CHUNK], BF16, tag="osb", name=f"o{i}")
        for m in range(KQ):
            ops = pp.tile([128, CHUNK], F32, tag="pp", name=f"op{i}_{m}")
            for kc in range(KQ // 2):
                nc.tensor.matmul(ops,
                                 wo_sb[:, 2 * kc:2 * kc + 2,
                                       m * 128:(m + 1) * 128],
                                 attnT_sb[:, 2 * kc:2 * kc + 2, :],
                                 start=(kc == 0), stop=False,
                                 perf_mode=DRMODE)
            # residual: PSUM += I.T @ featb  (scaled feat+bout, bf16)
            nc.tensor.matmul(ops, id_sb[:], fbb[:, m, :],
                             start=False, stop=True)
            nc.scalar.copy(out=o_sb[:, m, :], in_=ops)
            if m == 2:
                nc.sync.dma_start(out=out_l[b, :, 0:3, p0:p0 + CHUNK],
                                  in_=o_sb[:, 0:3, :])
        nc.sync.dma_start(out=out_l[b, :, 3:KQ, p0:p0 + CHUNK],
                          in_=o_sb[:, 3:KQ, :])

    def scoped(fn, tag, i):
        with nc.named_scope(f"{tag}{i}"):
            fn(i)

    scoped(stage_qproj, "qp", 0)
    scoped(stage_qk, "qk", 0)
    n = len(chunks)
    for i in range(n):
        scoped(stage_recip, "rc", i)
        if i + 1 < n:
            scoped(stage_qproj, "qp", i + 1)
        scoped(stage_avbc, "av", i)
        if i + 1 < n:
            scoped(stage_qk, "qk", i + 1)
        scoped(stage_oproj, "op", i)


_NC_CACHE = {}


def _get_nc():
    if "nc" not in _NC_CACHE:
        _NC_CACHE["nc"] = build(debug=False)
    return _NC_CACHE["nc"]


def _sigma_rows(w):
    """Swap 32-blocks within each 64-block along the last (out-channel) axis
    and keep sign (the eps sign lives in the skE/ckE tables)."""
    w4 = w.reshape(w.shape[0], w.shape[1] // 64, 2, 32)
    return np.ascontiguousarray(w4[:, :, ::-1, :].reshape(w.shape))


def _prep_in_maps(feat, tokens, Wq, Wkv, Wout, bout):
    feat = np.ascontiguousarray(feat, dtype=np.float32).reshape(B, DQ, NQ)
    tokens = np.ascontiguousarray(tokens, dtype=np.float32)
    bout = np.asarray(bout, dtype=np.float32)
    wk_full = Wkv[:, :DQ]
    shared = dict(
        wq8=np.ascontiguousarray(np.clip(
            (Wq * SW_Q).reshape(KQ, 128, DQ).transpose(1, 0, 2),
            -240, 240)).astype(NPF8),
        wk=np.ascontiguousarray(
            wk_full.reshape(KKV, 128, DQ).transpose(1, 0, 2)).astype(NPBF),
        wks=np.ascontiguousarray(
            _sigma_rows(wk_full).reshape(KKV, 128, DQ).transpose(1, 0, 2)
        ).astype(NPBF),
        wv=np.ascontiguousarray(
            (Wkv[:, DQ:] * SW_V).reshape(KKV, 128, DQ).transpose(1, 0, 2)
        ).astype(NPBF),
        wo8=np.ascontiguousarray(np.clip(
            (Wout * SW_O).reshape(KQ, 128, DQ).transpose(1, 0, 2),
            -240, 240)).astype(NPF8),
        **_consts(),
    )
    bout_r = bout.reshape(KQ, 128).transpose(1, 0)[None, :, :, None]
    in_maps = []
    for cid in range(NCORES):
        sl = slice(BL * cid, BL * (cid + 1))
        fl = np.ascontiguousarray(
            feat[sl].reshape(BL, KQ, 128, NQ).transpose(0, 2, 1, 3))
        tl = np.ascontiguousarray(tokens[sl].reshape(BL * NT, DKV)).astype(NPBF)
        in_maps.append(dict(
            featb=((fl + bout_r) * SW_VO).astype(NPBF),
            feat8=np.clip(fl, -240, 240).astype(NPF8),
            tok_l=tl, **shared))
    return in_maps


def _install_ntff_hook():
    """The container's antenv lacks axon_hooks; register the NTFF profile
    hook from trn_agent_boot ourselves so trace=True yields HW exec times."""
    import types

    import antenv
    from trn_agent_boot.trn_boot import _ntff_profile_via_ctypes

    mod = types.ModuleType("antenv.axon_hooks")
    state = {"hook": None}
    mod.set_axon_ntff_profile_hook = lambda h: state.__setitem__("hook", h)
    mod.get_axon_ntff_profile_hook = lambda: state["hook"]
    sys.modules["antenv.axon_hooks"] = mod
    antenv.axon_hooks = mod
    mod.set_axon_ntff_profile_hook(
        _ntff_profile_via_ctypes("/opt/axon/libaxon_pjrt.so"))
    # the S3 artifact upload has no credentials here; make it a no-op
    import concourse.bass_utils as bu
    bu.upload_artifacts = lambda tmpdir: f"local:{tmpdir}"


def run(inputs, trace=False, trace_cores=None):
    nc = _get_nc()
    if trace:
        try:
            _install_ntff_hook()
        except Exception as e:  # profiling is best-effort
            print(f"ntff hook install failed: {e}", file=sys.stderr)
            trace = False
    in_maps = _prep_in_maps(**inputs)
    res = run_bass_kernel_spmd(nc, in_maps, core_ids=list(range(NCORES)),
                               trace=trace, trace_cores=trace_cores)
    outs = []
    for r in res.results:
        ol = r["out_l"].astype(np.float32) * (1.0 / SW_VO)
        outs.append(ol.transpose(0, 2, 1, 3).reshape(BL, DQ, T, HP, WP))
    return np.ascontiguousarray(np.concatenate(outs, axis=0)), res


def kernel(**inputs):
    return run(inputs, trace=False)[0]
